# revision 33
# baseline (speedup 1.0000x reference)
"""LoRA linear kernel for 8 TRN2 NeuronCores.

Computes out = x @ (base_weight + SCALE * lora_B @ lora_A).T + bias
for x [4, 2048, 4096], base_weight [4096, 4096], rank 8.

Sharding (default mode 'f32r'): 2 token-halves x 4 d_out-quarters = 8
cores (tensor-parallel on d_out per the hint, plus a token split that
cuts per-core x traffic to ~100 MB vs ~160 MB for pure column-parallel).

Per core:
  - W' = W_q.T + SCALE * (A.T @ B_q.T) is materialized in SBUF once
    (rank-8 PE matmuls into PSUM + DVE adds), stored as 32 [128, O_CORE]
    k-tiles (~128 KB/partition). The LoRA product is never formed in HBM,
    so no extra weight traffic.
  - Main loop: for each 128-token tile, one contiguous 2 MB DMA loads the
    host-pretiled x.T block; its k-tiles are the stationary matmul operand
    and W' tiles the moving operand (fewest weight-loads per flop); 32
    accumulating matmuls per [128 x 512] PSUM tile; DVE adds the (free-dim)
    bias during PSUM->SBUF copyback; out rows DMA back in natural layout.
  - Matmuls run in float32r: 4-byte operands at full PE rate (1 cyc/row
    for moving dim >= 256), ~1.5e-4 relative error vs fp32 at K=4096 --
    measured end-to-end rel L2 error 1.8e-4.

Measured (slope over an on-device For_i repeat loop, 8 cores concurrent):
~550 us/pass; pure-matmul roofline 437 us, memory roofline ~280 us.
"""
import sys

if '/opt/trn_rl_repo' not in sys.path:
    sys.path.insert(0, '/opt/trn_rl_repo')

from contextlib import ExitStack

import numpy as np

import concourse.bacc as bacc
import concourse.mybir as mybir
import concourse.tile as tile
from concourse.bass_utils import run_bass_kernel_spmd

SCALE = 16.0 / 8.0  # alpha / rank

P = 128
K = 4096           # d_in (contraction)
KT = K // P        # 32 k-tiles
D_OUT = 4096
B, S = 4, 2048
T_FULL = B * S     # 8192 tokens
RANK = 8

# mode: (r_split, c_split, dt_x, dt_w)  [dt_x = stationary x, dt_w = moving W']
MODES = {
    'f32r':  (2, 4, 'f32r', 'f32r'),
    'f16':   (2, 4, 'f16',  'f16'),
    'wf16':  (4, 2, 'f32r', 'f16'),
    'f16c2': (4, 2, 'f16',  'f16'),
}
MM_DT = 'f32r'

R_SPLIT, C_SPLIT, DT_X, DT_W = MODES[MM_DT]
N_CORES = R_SPLIT * C_SPLIT
T_CORE = T_FULL // R_SPLIT
TT = T_CORE // P              # token tiles/core
O_CORE = D_OUT // C_SPLIT
OC = O_CORE // 512            # o-chunks of 512

_nc_cache = {}


def build_nc(repeat=1):
    """Build the per-core Bass program. `repeat` re-runs the main loop that
    many times inside a hardware loop (identical results; used for
    slope-based HW timing)."""
    if repeat in _nc_cache:
        return _nc_cache[repeat]
    f32 = mybir.dt.float32
    f32r = mybir.dt.float32r
    f16 = mybir.dt.float16
    dt_x = {'f32r': f32r, 'f16': f16}[DT_X]
    dt_w = {'f32r': f32r, 'f16': f16}[DT_W]
    cast_x = dt_x == f16
    cast_w = dt_w == f16

    nc = bacc.Bacc(None, target_bir_lowering=False)
    # x blocks: [t_tile, p(k-within-tile), kt, j(token-within-tile)]
    xb = nc.dram_tensor("xb", [TT, P, KT, P], f32 if cast_x else f32r,
                        kind="ExternalInput")
    wt = nc.dram_tensor("wt", [KT, P, O_CORE], f32 if cast_w else f32r,
                        kind="ExternalInput")
    a_in = nc.dram_tensor("a_in", [RANK, K], f32r, kind="ExternalInput")
    bts = nc.dram_tensor("bts", [RANK, O_CORE], f32r, kind="ExternalInput")
    biasb = nc.dram_tensor("biasb", [P, O_CORE], f32, kind="ExternalInput")
    out = nc.dram_tensor("out", [T_CORE, O_CORE], f32, kind="ExternalOutput")

    with ExitStack() as ctx:
        tc = ctx.enter_context(tile.TileContext(nc))
        wpool = ctx.enter_context(tc.tile_pool(name="wpool", bufs=1))
        cpool = ctx.enter_context(tc.tile_pool(name="cpool", bufs=1))
        apool = ctx.enter_context(tc.tile_pool(name="apool", bufs=2))
        nb = 2 if OC == 4 else 3
        xpool = ctx.enter_context(tc.tile_pool(name="xpool", bufs=nb))
        opool = ctx.enter_context(tc.tile_pool(name="opool", bufs=nb))
        pspool = ctx.enter_context(tc.tile_pool(name="ps", bufs=max(2, 6 // OC),
                                                space="PSUM"))
        # dedicated prep psum pool (OC==2 leaves 2 banks free); sharing tags
        # with the main psums would chain the intro tiles behind late prep
        pwpool = (ctx.enter_context(tc.tile_pool(name="psw", bufs=2,
                                                 space="PSUM"))
                  if OC == 2 else pspool)
        if cast_w or cast_x:
            spool = ctx.enter_context(tc.tile_pool(name="spool", bufs=2))

        # ---- constants / small tensors ----
        bts_t = cpool.tile([RANK, O_CORE], f32r, tag="bts")
        nc.sync.dma_start(bts_t[:], bts[:])
        bias_t = cpool.tile([P, O_CORE], f32, tag="bias")
        nc.sync.dma_start(bias_t[:], biasb[:])

        # ---- W' = W.T + SCALE*(A.T @ B.T), cached in SBUF as [k, o] ----
        # staging is done in 1024-wide pieces to bound SBUF usage; the same
        # slots are reused for x-cast staging in the main loop.
        WH = min(O_CORE, 1024)
        NH = O_CORE // WH
        wtiles = []
        wstage = {}
        for kt in range(KT):
            w_t = wpool.tile([P, O_CORE], dt_w, tag=f"w{kt}")
            if cast_w:
                for h in range(NH):
                    w_s = spool.tile([P, WH], f32, tag="stage",
                                     name=f"wstage_{kt}_{h}")
                    nc.sync.dma_start(w_s[:], wt[kt, :, h * WH:(h + 1) * WH])
                    wstage[(kt, h)] = w_s
            else:
                nc.sync.dma_start(w_t[:], wt[kt])
            wtiles.append(w_t)

        ACH = 4  # k-tiles of A per load chunk
        for ch in range(KT // ACH):
            a_sb = apool.tile([RANK, ACH * P], f32r)
            nc.sync.dma_start(a_sb[:], a_in[:, ch * ACH * P:(ch + 1) * ACH * P])
            for i in range(ACH):
                kt = ch * ACH + i
                for oc in range(OC):
                    psw = pwpool.tile([P, 512], f32, name=f"psw_{kt}_{oc}",
                                      tag="psw" if OC == 2 else f"ps{oc}")
                    nc.tensor.matmul(
                        psw[:],
                        a_sb[:, i * P:(i + 1) * P],
                        bts_t[:, oc * 512:(oc + 1) * 512],
                        start=True, stop=True,
                    )
                    sl = slice(oc * 512, (oc + 1) * 512)
                    if cast_w:
                        h = (oc * 512) // WH
                        hsl = slice(oc * 512 - h * WH, (oc + 1) * 512 - h * WH)
                        nc.vector.tensor_add(
                            wtiles[kt][:, sl], wstage[(kt, h)][:, hsl], psw[:])
                    else:
                        nc.vector.tensor_add(
                            wtiles[kt][:, sl],
                            wtiles[kt][:, sl].bitcast(f32),
                            psw[:])

        # ---- main loop: out[t, o] = x_tile.T @ W' (+ bias) ----
        def load_x(tt):
            xt = xpool.tile([P, KT, P], dt_x, name=f"xt_{tt}", tag="xt")
            if cast_x:
                # stage fp32 quarters, cast to fp16 on the (idle) ACT
                XQ = KT // 4
                for q in range(4):
                    x_s = spool.tile([P, XQ, P], f32, tag="stage",
                                     name=f"xs_{tt}_{q}")
                    nc.sync.dma_start(
                        x_s[:], xb[tt, :, q * XQ:(q + 1) * XQ, :])
                    nc.scalar.copy(
                        xt[:, q * XQ:(q + 1) * XQ, :], x_s[:])
            else:
                nc.sync.dma_start(xt[:], xb[tt])
            return xt

        def alloc_ps(tt):
            return [pspool.tile([P, 512], f32, tag=f"ps{oc}",
                                name=f"ps_{tt}_{oc}")
                    for oc in range(OC)]

        def flush(tt, pss):
            o_t = opool.tile([P, O_CORE], f32, name=f"ot_{tt}", tag="ot")
            for oc in range(OC):
                sl = slice(oc * 512, (oc + 1) * 512)
                nc.vector.tensor_add(o_t[:, sl], pss[oc][:], bias_t[:, sl])
            nc.sync.dma_start(out[tt * P:(tt + 1) * P, :], o_t[:])

        # First INTRO token tiles are interleaved k-major so the PE consumes
        # each W' k-tile INTRO*OC times as it streams in from HBM, hiding the
        # W-load ramp. INTRO*OC PSUM banks stay live, so INTRO*OC <= 6.
        INTRO = min(TT, 6 // OC)

        def main_pass(intro):
            if intro:
                ixt = [load_x(tt) for tt in range(INTRO)]
                ips = [alloc_ps(tt) for tt in range(INTRO)]
                for k in range(KT):
                    for tt in range(INTRO):
                        for oc in range(OC):
                            nc.tensor.matmul(
                                ips[tt][oc][:],
                                ixt[tt][:, k, :],
                                wtiles[k][:, oc * 512:(oc + 1) * 512],
                                start=(k == 0), stop=(k == KT - 1),
                            )
                for tt in range(INTRO):
                    flush(tt, ips[tt])
                start_tt = INTRO
            else:
                start_tt = 0
            for tt in range(start_tt, TT):
                xt = load_x(tt)
                pss = alloc_ps(tt)
                for k in range(KT):
                    for oc in range(OC):
                        nc.tensor.matmul(
                            pss[oc][:],
                            xt[:, k, :],
                            wtiles[k][:, oc * 512:(oc + 1) * 512],
                            start=(k == 0), stop=(k == KT - 1),
                        )
                flush(tt, pss)

        if repeat == 1:
            main_pass(intro=True)
        else:
            with tc.For_i(0, repeat, 1):
                main_pass(intro=True)

    nc.compile()
    _nc_cache[repeat] = nc
    return nc


def _prep_in_maps(x, base_weight, lora_A, lora_B, bias):
    x2d = np.ascontiguousarray(x.reshape(T_FULL, K), dtype=np.float32)
    WT = np.ascontiguousarray(base_weight.T.astype(np.float32, copy=False))
    BTs = np.ascontiguousarray((SCALE * lora_B).T.astype(np.float32, copy=False))
    a_np = np.ascontiguousarray(lora_A.astype(np.float32, copy=False))
    bias = bias.astype(np.float32, copy=False)

    xbs = []
    for h in range(R_SPLIT):
        xh = x2d[h * T_CORE:(h + 1) * T_CORE]
        # [tt, j(tok), kt, p(k)] -> [tt, p, kt, j]
        xb = np.ascontiguousarray(
            xh.reshape(TT, P, KT, P).transpose(0, 3, 2, 1))
        xbs.append(xb)

    in_maps = []
    for h in range(R_SPLIT):
        for q in range(C_SPLIT):
            osl = slice(q * O_CORE, (q + 1) * O_CORE)
            wt = np.ascontiguousarray(WT[:, osl]).reshape(KT, P, O_CORE)
            bts = np.ascontiguousarray(BTs[:, osl])
            biasb = np.ascontiguousarray(
                np.broadcast_to(bias[osl][None, :], (P, O_CORE)))
            in_maps.append({
                "xb": xbs[h], "wt": wt, "a_in": a_np,
                "bts": bts, "biasb": biasb,
            })
    return in_maps


def _assemble(results):
    flat = np.empty((T_FULL, D_OUT), dtype=np.float32)
    i = 0
    for h in range(R_SPLIT):
        for q in range(C_SPLIT):
            flat[h * T_CORE:(h + 1) * T_CORE,
                 q * O_CORE:(q + 1) * O_CORE] = results[i]["out"]
            i += 1
    return flat.reshape(B, S, D_OUT)


def kernel(x, base_weight, lora_A, lora_B, bias):
    x = np.asarray(x)
    base_weight = np.asarray(base_weight)
    lora_A = np.asarray(lora_A)
    lora_B = np.asarray(lora_B)
    bias = np.asarray(bias)
    nc = build_nc()
    in_maps = _prep_in_maps(x, base_weight, lora_A, lora_B, bias)
    res = run_bass_kernel_spmd(nc, in_maps, core_ids=list(range(N_CORES)))
    return _assemble(res.results)
